# revision 75
# baseline (speedup 1.0000x reference)
"""BSBR attention kernel for 8 Trainium2 NeuronCores.

Sharding: data-parallel over batch (B=2) x tensor-parallel over heads
(16 heads -> 4 heads per core). Core c handles batch c//4, head group c%4.
Each core computes its 4 heads' attention output and the partial output
projection (attn_heads @ Wo[:, head_dims].T); the host sums the 4 partials
per batch and adds bo.

Device-side design notes:
- K is projected ONCE (natural [s, dh] layout, fused with V in one N=512
  chain); K^T [dh, s] is derived by DMA XBAR transposes through a DRAM
  bounce in quarters that hide under the projection matmuls.
- V/K biases are applied during PSUM evacuation via a host-prebroadcast
  [128, 512] bias tile (tensor_add), not rank-1 matmuls. Weights arrive
  host-prearranged so every weight load is one contiguous DMA.
- DMA discipline (the big lever on this part): DGE queues are strict
  FIFO and each dma_start costs ~0.7us of the issuing engine's
  SEQUENCER, serializing with its compute dispatch. So the Sync queue
  (no compute) carries x + every mid-kernel bounce + outputs, ordered so
  readiness is monotone; Scalar's queue only has startup weight loads.
- The F/retrieval layout bounces are descriptor-bound (~2k 128B
  descriptors per quarter for each d<->c swap). F for chunks 0-15 runs
  as a dense block at the projection midpoint and each bounce puts the
  128B-granular side on the WRITE, so the descriptor work streams while
  the projections/scores run and the reads stay contiguous.
- Scores loop per chunk: S^T = K Q^T (row-pair-packed), exp on ACT, mask
  on DVE, U (+ones column) matmul, row-recips; raw U + recips persist.
  The chunk-probs mix is spread one N=512 slice per scores iteration.
- Tail is a 3-stage software pipeline per chunk: LT matmuls + U*recip+LT
  combine (scalar_tensor_tensor on DVE); PE transposes + attnT copies a
  chunk ahead (so LDWEIGHTS prefetches); output projection with ACT
  evacuation and bf16 output DMA spread across the whole tail.
- Output is written bf16; the host sums partials in f32 and adds bo.
"""

import numpy as np

try:
    import concourse.bass as bass
except ImportError:
    import sys

    sys.path.insert(0, "/opt/trn_rl_repo")
    import concourse.bass as bass

import ml_dtypes
from contextlib import ExitStack

import concourse.tile as tile
from concourse import mybir
from concourse.bass_utils import run_bass_kernel_spmd

BF16 = ml_dtypes.bfloat16
B, S, D, H, CS = 2, 4096, 1024, 16, 128
HD = D // H          # 64
C = S // CS          # 32
NCORES = 8
DHC = 4 * HD         # 256 head dims per core
KB = D // 128        # 8 contraction blocks
NEG = -1e9

bf = mybir.dt.bfloat16
f32 = mybir.dt.float32
Exp = mybir.ActivationFunctionType.Exp
MULT = mybir.AluOpType.mult
ADD = mybir.AluOpType.add


def _split_heavy_waits(nc, keep=1):
    """The walrus build in this container rejects >keep sync waits on several
    instruction encodings. Hoist excess waits onto preceding NoOps on the
    same engine — the sequencer executes them in order."""
    for fn in nc.m.functions:
        for bb in fn.blocks:
            insts = bb.instructions
            i = 0
            while i < len(insts):
                inst = insts[i]
                si = inst.sync_info
                if si is not None and si.on_wait and len(si.on_wait) > keep:
                    waits = list(si.on_wait)
                    head, tail = waits[:-keep], waits[-keep:]
                    for j, w in enumerate(head):
                        nop = mybir.InstNoOp(
                            name=nc.get_next_instruction_name(), ins=[], outs=[]
                        )
                        nop.engine = inst.engine
                        nop.sync_info = mybir.SyncInfo(on_wait=[w], on_update=[])
                        nc.register_instruction(nop, overwrite=True)
                        insts.insert(i + j, nop)
                    inst.sync_info = mybir.SyncInfo(
                        on_wait=tail, on_update=list(si.on_update)
                    )
                    i += len(head)
                i += 1


def _build_program():
    nc = bass.Bass("TRN2", debug=False, num_devices=NCORES)

    ap = {}
    def din(name, shape, dtype):
        ap[name] = nc.dram_tensor(name, shape, dtype, kind="ExternalInput").ap()

    din("xT", [D, S], bf)
    # weights arrive pre-arranged [p, k, d] (host does the permute) so each
    # load is one contiguous-per-partition DMA
    for w in ("wqT", "wrT", "whT"):
        din(w, [128, KB, DHC], bf)
    din("wvkT", [128, KB, 512], bf)
    din("woT", [128, 2, D], bf)
    din("bias", [128, 6], f32)
    din("bvb", [128, 512], bf)      # [bv | bk] broadcast to 128 partitions
    din("ident", [128, 128], bf)
    din("maskb4", [128, 512], bf)   # binary local mask, [t, s] tiled x4 heads
    din("cmask4", [128, C], f32)    # additive chunk mask [c', c] tiled x4
    out_ap = nc.dram_tensor("out", [S, D], bf, kind="ExternalOutput").ap()

    with tile.TileContext(nc) as tc, ExitStack() as ctx:
        const = ctx.enter_context(tc.tile_pool(name="const", bufs=1))
        wpool = ctx.enter_context(tc.tile_pool(name="wpool", bufs=1))
        big = ctx.enter_context(tc.tile_pool(name="big", bufs=1))
        dram = ctx.enter_context(tc.tile_pool(name="dramp", bufs=1, space="DRAM"))

        # ---- weights first (smallest latency to first matmul) ----
        # DMA instruction issue costs ~0.7us of the issuing engine's
        # sequencer and serializes with its compute dispatch. So: wq + the
        # x stream + all mid-kernel DMAs go on Sync (which has no compute);
        # Scalar's queue only carries startup weight/const loads that
        # complete before ACT's first bias add.
        def load_w(name, eng, shape):
            t = wpool.tile(shape, bf, name=f"{name}_sb")
            eng.dma_start(t[:], ap[name][:])
            return t

        # sync queue carries only x (and later feedback DMAs); weights ride
        # scalar so the first matmul's two inputs stream in parallel
        wq_sb = load_w("wqT", nc.scalar, [128, KB, DHC])
        wvk_sb = load_w("wvkT", nc.scalar, [128, KB, 512])
        bvb_sb = const.tile([128, 512], bf)
        nc.scalar.dma_start(bvb_sb[:], ap["bvb"][:])
        bias_sb = const.tile([128, 6], f32)
        nc.scalar.dma_start(bias_sb[:], ap["bias"][:])

        # ---- persistent activations ----
        qt_sb = [big.tile([128, S], bf, name=f"qt{m}") for m in range(2)]
        kt_sb = big.tile([128, 2, S], bf)        # [d-in-m, m, s]
        v_sb = [big.tile([128, 4 * 65], bf, name=f"v{i}") for i in range(C)]
        knat_sb = big.tile([128, C, DHC], bf)    # [s-in-chunk, chunk, d]
        rt_sb = big.tile([128, 2, C], bf)
        ht_sb = big.tile([128, 2, C], bf)
        expct_sb = big.tile([128, C], bf)
        crecip_sb = big.tile([128, 1], f32)
        fnat_sb = [big.tile([128, C * 64], bf, name=f"fnat{p}") for p in range(2)]
        frows_sb = big.tile([128, 64 * 64], bf)
        fb = dram.tile([2, 2, C, 64, 64], bf)    # (pair, h2, c', d, e) c-major
        rbt = dram.tile([4, C, 64, 64], bf)      # (head, c, d, e)
        knd = dram.tile([C, 128, DHC], bf)       # K natural bounce for XBAR T

        onescol_sb = const.tile([128, 1], bf)
        nc.vector.memset(onescol_sb[:], 1.0)
        # ones column per head inside each v tile (static, written once)
        for i in range(C):
            nc.vector.memset(
                v_sb[i].rearrange("p (h e) -> p h e", e=65)[:, :, 64:65], 1.0
            )

        # ---- phase 1: load x, projections ----
        with tc.tile_pool(name="xtp", bufs=1) as xtpool:
            xt_sb = xtpool.tile([128, KB, S], bf)
            # one DMA instruction per s-slice (all k blocks in its AP): the
            # first projection matmuls can start after ~1/8 of x arrives.
            # n=0 is further split so the first chain's k=0..3 deps land
            # ~3us earlier.
            xsrc = ap["xT"].rearrange("(k p) s -> p k s", p=128)
            nc.sync.dma_start(xt_sb[:, 0:4, 0:512], xsrc[:, 0:4, 0:512])
            nc.sync.dma_start(xt_sb[:, 4:8, 0:512], xsrc[:, 4:8, 0:512])
            nc.scalar.dma_start(xt_sb[:, 4:8, 512:1024], xsrc[:, 4:8, 512:1024])
            nc.sync.dma_start(xt_sb[:, 0:4, 512:1024], xsrc[:, 0:4, 512:1024])
            for n in range(2, 8):
                nc.sync.dma_start(
                    xt_sb[:, :, n * 512 : (n + 1) * 512],
                    xsrc[:, :, n * 512 : (n + 1) * 512],
                )

            # low-urgency weights/consts after the x stream, scalar queue
            wr_sb = load_w("wrT", nc.scalar, [128, KB, DHC])
            wh_sb = load_w("whT", nc.scalar, [128, KB, DHC])
            wo_sb = load_w("woT", nc.scalar, [128, 2, D])
            maskb_sb = const.tile([128, 512], bf)
            nc.scalar.dma_start(maskb_sb[:], ap["maskb4"][:])
            cmask_sb = const.tile([128, C], f32)
            nc.scalar.dma_start(cmask_sb[:], ap["cmask4"][:])
            ident_sb = const.tile([128, 128], bf)
            nc.scalar.dma_start(ident_sb[:], ap["ident"][:])

            bvbv = bvb_sb[:, 0:DHC].rearrange("p (h e) -> p h e", e=64)
            knd_pc = knd.rearrange("c p d -> p c d")
            knd_flat = knd.rearrange("c p d -> (c p) d")

            def emit_k_bounce(q):
                # K^T via DRAM bounce + XBAR transpose (no PE cost), in
                # quarters so the transpose engine time hides under the
                # projection matmuls. Rides the (otherwise idle) sync queue.
                c0, c1 = q * 8, q * 8 + 8
                nc.sync.dma_start(
                    knd_pc[:, c0:c1, :], knat_sb[:, c0:c1, :]
                )
                for m in range(2):
                    nc.sync.dma_start_transpose(
                        kt_sb[:, m, c0 * 128 : c1 * 128],
                        knd_flat[c0 * 128 : c1 * 128, m * 128 : (m + 1) * 128],
                    )

            def emit_f_block(fpsp, chunks):
                # F = k^T v for chunks whose knat/v landed last iteration
                for i in chunks:
                    vr = v_sb[i].rearrange("p (h e) -> p h e", e=65)
                    for p in range(2):
                        fps = fpsp.tile([128, 64], f32, tag="fps")
                        for h2 in range(2):
                            nc.tensor.matmul(
                                fps[64 * h2 : 64 * h2 + 64, :],
                                knat_sb[:, i, (2 * p + h2) * 64 : (2 * p + h2) * 64 + 64],
                                vr[:, 2 * p + h2, 0:64],
                                start=True, stop=True, skip_group_check=True,
                            )
                        if (i + p) % 2 == 0:
                            nc.vector.tensor_copy(
                                fnat_sb[p][:, i * 64 : (i + 1) * 64], fps[:]
                            )
                        else:
                            nc.scalar.copy(
                                fnat_sb[p][:, i * 64 : (i + 1) * 64], fps[:]
                            )

            def emit_f_bounce(q):
                # bounce F quarter to row layout (sync queue). The d<->c swap
                # costs ~2k 128B descriptors per quarter no matter what; put
                # that slow side on the WRITE (c-major dst) so it overlaps F
                # and the scores loop, leaving a fast contiguous read.
                c0, c1 = q * 8, q * 8 + 8
                for p in range(2):
                    for h2 in range(2):
                        nc.sync.dma_start(
                            fb[p, h2][c0:c1].rearrange("c d e -> d c e"),
                            fnat_sb[p][64 * h2 : 64 * h2 + 64, c0 * 64 : c1 * 64]
                            .rearrange("d (c e) -> d c e", e=64),
                        )

            def emit_f_rows_read():
                for h in range(4):
                    p, h2 = divmod(h, 2)
                    nc.sync.dma_start(
                        frows_sb[32 * h : 32 * h + 32, :],
                        fb[p, h2].rearrange("c d e -> c (d e)"),
                    )

            with (
                tc.tile_pool(name="pjp", bufs=2, space="PSUM") as pjp,
                tc.tile_pool(name="fps", bufs=2, space="PSUM") as fpsp,
            ):
                for n in range(8):
                    if n == 4:
                        # F for the first half as one dense block: its slow
                        # row-layout bounce (128B descriptors) streams while
                        # the second half of the projections runs
                        for q in range(2):
                            emit_f_block(fpsp, range(q * 8, q * 8 + 8))
                            emit_f_bounce(q)
                    # QT: [dh, s] layout
                    for m in range(2):
                        ps = pjp.tile([128, 512], f32, tag="pj")
                        for k in range(KB):
                            nc.tensor.matmul(
                                ps[:],
                                wq_sb[:, k, m * 128 : (m + 1) * 128],
                                xt_sb[:, k, n * 512 : (n + 1) * 512],
                                start=(k == 0),
                                stop=(k == KB - 1),
                            )
                        nc.scalar.add(
                            qt_sb[m][:, n * 512 : (n + 1) * 512],
                            ps[:],
                            bias_sb[:, m : m + 1],
                        )
                    # V + K natural [s, dh] in one N=512 matmul chain; biases
                    # added during PSUM evacuation from a prebroadcast tile
                    for i in range(4 * n, 4 * n + 4):
                        ps = pjp.tile([128, 512], f32, tag="pv")
                        for k in range(KB):
                            nc.tensor.matmul(
                                ps[:],
                                xt_sb[:, k, i * 128 : (i + 1) * 128],
                                wvk_sb[:, k, :],
                                start=(k == 0),
                                stop=(k == KB - 1),
                                skip_group_check=True,
                            )
                        vr = v_sb[i].rearrange("p (h e) -> p h e", e=65)
                        nc.vector.tensor_add(
                            vr[:, :, 0:64],
                            ps[:, 0:DHC].rearrange("p (h e) -> p h e", e=64),
                            bvbv,
                        )
                        nc.vector.tensor_add(
                            knat_sb[:, i, :], ps[:, DHC:512], bvb_sb[:, DHC:512]
                        )
                    if n % 2 == 1:
                        emit_k_bounce(n // 2)

                # r/h meta projections: [dh, c] layout
                crepr = [
                    xt_sb[:, k, :].rearrange("p (c cs) -> p c cs", cs=CS)[:, :, CS - 1]
                    for k in range(KB)
                ]
                for w_sb, dst, bcol in ((wr_sb, rt_sb, 2), (wh_sb, ht_sb, 4)):
                    for m in range(2):
                        ps = pjp.tile([128, C], f32, tag="pr")
                        for k in range(KB):
                            nc.tensor.matmul(
                                ps[:],
                                w_sb[:, k, m * 128 : (m + 1) * 128],
                                crepr[k],
                                start=(k == 0),
                                stop=(k == KB - 1),
                            )
                        nc.scalar.add(
                            dst[:, m, :], ps[:], bias_sb[:, bcol + m : bcol + m + 1]
                        )

        # phase-2+ tensors reuse the space freed by the xt tiles
        anp2 = ctx.enter_context(tc.tile_pool(name="anp2", bufs=1))
        retrrows_sb = anp2.tile([128, 64 * 64], bf)
        retrt_sb = [anp2.tile([128, C * 64], bf, name=f"retrt{p}") for p in range(2)]
        ubf_sb = [anp2.tile([128, DHC], bf, name=f"ubf{i}") for i in range(C)]
        rr_sb = [anp2.tile([128, 4], f32, name=f"rr{i}") for i in range(C)]

        # ---- F = k^T v per chunk (dense standalone phase right after the
        # projections), with the slow row-layout bounce kicked off per
        # 8-chunk quarter so the gathers overlap the rest of F + scores ----
        with tc.tile_pool(name="fps", bufs=4, space="PSUM") as fpsp:
            for q in range(2, 4):
                emit_f_block(fpsp, range(q * 8, q * 8 + 8))
                emit_f_bounce(q)
            emit_f_rows_read()

        # chunk scores -> probs; crecip (per query chunk) applied on mix output
        with tc.tile_pool(name="csp", bufs=1, space="PSUM") as cspp:
            csp = cspp.tile([128, C], f32, tag="csp")
            for h in range(4):
                hb = 64 * (h % 2)
                nc.tensor.matmul(
                    csp[32 * h : 32 * h + 32, :],
                    ht_sb[hb : hb + 64, h // 2, :],
                    rt_sb[hb : hb + 64, h // 2, :],
                    start=True, stop=True, skip_group_check=True,
                    tile_position=(hb, 32 * h),
                )
            nc.vector.tensor_add(csp[:], csp[:], cmask_sb[:])
            nc.scalar.activation(expct_sb[:], csp[:], Exp, scale=0.125)
            csums = cspp.tile([128, 1], f32, tag="csums")
            for h in range(4):
                nc.tensor.matmul(
                    csums[32 * h : 32 * h + 32, :],
                    expct_sb[32 * h : 32 * h + 32, :],
                    onescol_sb[32 * h : 32 * h + 32, :],
                    start=True, stop=True, skip_group_check=True,
                    tile_position=(32 * h, 32 * h),
                )
            nc.vector.reciprocal(crecip_sb[:], csums[:])

        # ---- scores loop (st/exp/U) overlapping the retrieval bounces ----
        # Concurrent (row-disjoint) PE matmuls must not drain into the
        # same PSUM bank on the same partitions: heads with operands at
        # base 0 (h0/h2) and base 64 (h1/h3) can execute concurrently on
        # disjoint PE row groups, so each group gets its own bank.
        POS = {0: (0, 0), 2: (0, 1), 1: (1, 0), 3: (1, 1)}
        HORD = (0, 2, 1, 3)

        with (
            tc.tile_pool(name="stp", bufs=2, space="PSUM") as stp,
            tc.tile_pool(name="ulp", bufs=2, space="PSUM") as ulp,
            tc.tile_pool(name="exps", bufs=4) as expp,
            tc.tile_pool(name="smalls", bufs=4) as smalls,
        ):
            expst_q = {}

            def emit_st(i):
                st = stp.tile([128, 1024], f32, tag="st")
                for h in HORD:
                    hp, hb = h // 2, 64 * (h % 2)
                    g, b = POS[h]
                    nc.tensor.matmul(
                        st[:, g * 512 + b * 128 : g * 512 + (b + 1) * 128],
                        kt_sb[hb : hb + 64, hp, i * 128 : (i + 1) * 128],
                        qt_sb[hp][hb : hb + 64, i * 128 : (i + 1) * 128],
                        start=(b == 0), stop=(b == 1), skip_group_check=True,
                    )
                stv = st.rearrange("p (g c) -> p g c", c=512)[:, :, 0:256].rearrange(
                    "p g (b e) -> p g b e", e=128
                )
                expraw = expp.tile([128, 512], bf, tag="expraw")
                nc.scalar.activation(expraw[:], stv, Exp, scale=0.125)
                expst = expp.tile([128, 512], bf, tag="expst")
                nc.vector.tensor_mul(expst[:], expraw[:], maskb_sb[:])
                expst_q[i] = expst

            def emit_u(i):
                # trails emit_st by one chunk so the U matmuls' stationary
                # operand (expst) is ready early enough for LDWEIGHTS
                # prefetch
                expst = expst_q.pop(i)
                u = ulp.tile([128, 4 * 65], f32, tag="u")
                for h in range(4):
                    g, b = POS[h]
                    pos = g * 2 + b
                    nc.tensor.matmul(
                        u[:, h * 65 : (h + 1) * 65],
                        expst[:, pos * 128 : (pos + 1) * 128],
                        v_sb[i][:, h * 65 : (h + 1) * 65],
                        start=(h == 0), stop=(h == 3), skip_group_check=True,
                    )
                # row-sum reciprocals straight from the PSUM ones column;
                # U is stored raw (normalization folds into the tail combine)
                nc.vector.reciprocal(
                    rr_sb[i][:], u.rearrange("p (h e) -> p h e", e=65)[:, :, 64]
                )
                nc.vector.tensor_copy(
                    ubf_sb[i].rearrange("p (h e) -> p h e", e=64),
                    u.rearrange("p (h e) -> p h e", e=65)[:, :, 0:64],
                )

            for i in range(C):
                emit_st(i)
                if i >= 1:
                    emit_u(i - 1)
                if i == C - 1:
                    emit_u(i)
                if 6 <= i < 14:
                    # retrieved rows = probs @ F rows, one N=512 slice per
                    # scores iteration (spreads PE/ACT/DVE load and rides
                    # out the F-bounce latency)
                    nb = i - 6
                    mps = ulp.tile([128, 512], f32, tag="mps")
                    for h in range(4):
                        nc.tensor.matmul(
                            mps[32 * h : 32 * h + 32, :],
                            expct_sb[32 * h : 32 * h + 32, :],
                            frows_sb[32 * h : 32 * h + 32, nb * 512 : (nb + 1) * 512],
                            start=True, stop=True, skip_group_check=True,
                            tile_position=(32 * h, 32 * h),
                        )
                    if nb % 2 == 0:
                        nc.scalar.mul(
                            retrrows_sb[:, nb * 512 : (nb + 1) * 512],
                            mps[:],
                            crecip_sb[:, 0:1],
                        )
                    else:
                        nc.vector.tensor_scalar_mul(
                            retrrows_sb[:, nb * 512 : (nb + 1) * 512],
                            mps[:],
                            crecip_sb[:, 0:1],
                        )
                if i == 14:
                    # retrieved rows -> retrT via DRAM bounce (sync queue is
                    # idle between the K^T transposes and the output writes).
                    # The reads come back per 8-chunk quarter so the tail's
                    # first chunks unblock ~3 gathers earlier.
                    for h in range(4):
                        nc.sync.dma_start(
                            rbt[h],
                            retrrows_sb[32 * h : 32 * h + 32, :].rearrange(
                                "c (d e) -> c d e", e=64
                            ),
                        )
                    for cq in range(4):
                        c0, c1 = cq * 8, cq * 8 + 8
                        for p in range(2):
                            for h2 in range(2):
                                nc.sync.dma_start(
                                    retrt_sb[p][
                                        64 * h2 : 64 * h2 + 64, c0 * 64 : c1 * 64
                                    ].rearrange("d (c e) -> d c e", e=64),
                                    rbt[2 * p + h2][c0:c1].rearrange("c d e -> d c e"),
                                )

        # ---- tail loop: LT + combine for chunk i, transpose + output
        # projection + DMA for chunk i-1 (software-pipelined so the PE never
        # waits on the DVE combine or the attnT evacuation round trips) ----
        with (
            tc.tile_pool(name="ltp", bufs=2, space="PSUM") as ltp,
            tc.tile_pool(name="tpp", bufs=2, space="PSUM") as tpp,
            tc.tile_pool(name="outp", bufs=2, space="PSUM") as outp,
            tc.tile_pool(name="attns", bufs=4) as attns,
            tc.tile_pool(name="outs", bufs=4) as outs,
        ):
            attn_q = {}
            attnT_q = {}

            def emit_lt(i):
                lt = ltp.tile([128, 1024], f32, tag="lt")
                for h in HORD:
                    hp, hb = h // 2, 64 * (h % 2)
                    g, b = POS[h]
                    nc.tensor.matmul(
                        lt[:, g * 512 + b * 64 : g * 512 + (b + 1) * 64],
                        qt_sb[hp][hb : hb + 64, i * 128 : (i + 1) * 128],
                        retrt_sb[hp][hb : hb + 64, i * 64 : (i + 1) * 64],
                        start=(b == 0), stop=(b == 1), skip_group_check=True,
                    )
                # attn = U * rowrecip + LT per head on DVE
                attn = attns.tile([128, DHC], bf, tag="attn")
                ur = ubf_sb[i].rearrange("p (h e) -> p h e", e=64)
                for h in range(4):
                    g, b = POS[h]
                    nc.vector.scalar_tensor_tensor(
                        attn[:, h * 64 : (h + 1) * 64],
                        ur[:, h, :],
                        rr_sb[i][:, h : h + 1],
                        lt[:, g * 512 + b * 64 : g * 512 + (b + 1) * 64],
                        op0=MULT,
                        op1=ADD,
                    )
                attn_q[i] = attn

            def emit_transpose(j):
                attn = attn_q.pop(j)
                attnT = attns.tile([128, 2, 128], bf, tag="attnT")
                for half in range(2):
                    tp = tpp.tile([128, 128], bf, tag="tp")
                    nc.tensor.transpose(
                        tp[:], attn[:, half * 128 : (half + 1) * 128], ident_sb[:]
                    )
                    nc.vector.tensor_copy(attnT[:, half, :], tp[:])
                attnT_q[j] = attnT

            def emit_out(j):
                attnT = attnT_q.pop(j)
                osb = outs.tile([128, D], bf, tag="osb")
                for nb in range(2):
                    ops = outp.tile([128, 512], f32, tag="ops")
                    for p in range(2):
                        nc.tensor.matmul(
                            ops[:],
                            attnT[:, p, :],
                            wo_sb[:, p, nb * 512 : (nb + 1) * 512],
                            start=(p == 0),
                            stop=(p == 1),
                        )
                    if nb == 0 and j >= C - 2:
                        # drain: nothing follows, use both engines in parallel
                        nc.vector.tensor_copy(osb[:, 0:512], ops[:])
                    else:
                        nc.scalar.copy(osb[:, nb * 512 : (nb + 1) * 512], ops[:])
                nc.sync.dma_start(out_ap[j * 128 : (j + 1) * 128, :], osb[:])

            # 3-stage pipeline: the attnT copies lead their output projection
            # by a full chunk so the PE's weight load can prefetch
            for i in range(C):
                emit_lt(i)
                if i >= 1:
                    emit_transpose(i - 1)
                if i >= 2:
                    emit_out(i - 2)
            emit_transpose(C - 1)
            emit_out(C - 2)
            emit_out(C - 1)

    _split_heavy_waits(nc)
    return nc


_CACHE = {}


def _get_program():
    if "nc" not in _CACHE:
        _CACHE["nc"] = _build_program()
    return _CACHE["nc"]


def _make_in_maps(inputs):
    hs = np.asarray(inputs["hidden_states"], dtype=np.float32)
    W = {k: np.asarray(inputs[k], dtype=np.float32) for k in
         ("Wq", "Wk", "Wv", "Wo", "Wr", "Wh")}
    bvec = {k: np.asarray(inputs[k], dtype=np.float32) for k in
            ("bq", "bk", "bv", "bo", "br", "bh")}

    # local binary mask in [t, s] layout (keep t >= s), tiled x4 heads
    tt, ss = np.meshgrid(np.arange(128), np.arange(128), indexing="ij")
    maskb = (tt >= ss).astype(np.float32)
    maskb4 = np.tile(maskb, (1, 4)).astype(BF16)
    # chunk mask in [c', c] layout: keep c' >= c; tiled x4 heads (additive)
    cc2, cc = np.meshgrid(np.arange(C), np.arange(C), indexing="ij")
    cmask = np.where(cc2 >= cc, 0.0, NEG).astype(np.float32)
    cmask4 = np.tile(cmask, (4, 1)).astype(np.float32)
    ident = np.eye(128, dtype=np.float32).astype(BF16)

    xT_bf = [np.ascontiguousarray(hs[b].T).astype(BF16) for b in range(B)]

    in_maps = []
    for c in range(NCORES):
        b, hg = divmod(c, 4)
        sl = slice(hg * DHC, (hg + 1) * DHC)
        bias = np.stack(
            [
                bvec["bq"][sl][:128], bvec["bq"][sl][128:],
                bvec["br"][sl][:128], bvec["br"][sl][128:],
                bvec["bh"][sl][:128], bvec["bh"][sl][128:],
            ],
            axis=1,
        ).astype(np.float32)
        bvb = np.tile(
            np.concatenate([bvec["bv"][sl], bvec["bk"][sl]])[None, :], (128, 1)
        ).astype(BF16)

        # pre-arrange weights to the device's [p, k, d] layout so each load
        # is a single contiguous-per-partition DMA
        def arr(wT):  # wT: [D, cols] -> [128, KB, cols]
            return np.ascontiguousarray(
                wT.reshape(KB, 128, wT.shape[1]).transpose(1, 0, 2)
            ).astype(BF16)

        wvk = np.concatenate([W["Wv"][sl, :].T, W["Wk"][sl, :].T], axis=1)
        woT = W["Wo"][:, sl].T  # [DHC, D]
        in_maps.append(
            {
                "xT": xT_bf[b],
                "wqT": arr(W["Wq"][sl, :].T),
                "wvkT": arr(wvk),
                "wrT": arr(W["Wr"][sl, :].T),
                "whT": arr(W["Wh"][sl, :].T),
                "woT": np.ascontiguousarray(
                    woT.reshape(2, 128, D).transpose(1, 0, 2)
                ).astype(BF16),
                "bias": bias,
                "bvb": bvb,
                "ident": ident,
                "maskb4": maskb4,
                "cmask4": cmask4,
            }
        )
    return in_maps, bvec["bo"]


def kernel(**inputs):
    nc = _get_program()
    in_maps, bo = _make_in_maps(inputs)
    res = run_bass_kernel_spmd(nc, in_maps, core_ids=list(range(NCORES)))
    _CACHE["last_results"] = res
    out = np.zeros((B, S, D), np.float32)
    for c in range(NCORES):
        out[c // 4] += res.results[c]["out"].astype(np.float32)
    out += bo[None, None, :]
    return out


# revision 76
# speedup vs baseline: 1.0357x; 1.0357x over previous
"""BSBR attention kernel for 8 Trainium2 NeuronCores.

Sharding: data-parallel over batch (B=2) x tensor-parallel over heads
(16 heads -> 4 heads per core). Core c handles batch c//4, head group c%4.
Each core computes its 4 heads' attention output and the partial output
projection (attn_heads @ Wo[:, head_dims].T); the host sums the 4 partials
per batch and adds bo.

Device-side design notes:
- K is projected ONCE (natural [s, dh] layout, fused with V in one N=512
  chain); K^T [dh, s] is derived by DMA XBAR transposes through a DRAM
  bounce in quarters that hide under the projection matmuls.
- V/K biases are applied during PSUM evacuation via a host-prebroadcast
  [128, 512] bias tile (tensor_add), not rank-1 matmuls. Weights arrive
  host-prearranged so every weight load is one contiguous DMA.
- DMA discipline (the big lever on this part): DGE queues are strict
  FIFO and each dma_start costs ~0.7us of the issuing engine's
  SEQUENCER, serializing with its compute dispatch. So the Sync queue
  (no compute) carries x + every mid-kernel bounce + outputs, ordered so
  readiness is monotone; Scalar's queue only has startup weight loads.
- The F/retrieval layout bounces are descriptor-bound (~2k 128B
  descriptors per quarter for each d<->c swap). F for chunks 0-15 runs
  as a dense block at the projection midpoint and each bounce puts the
  128B-granular side on the WRITE, so the descriptor work streams while
  the projections/scores run and the reads stay contiguous.
- Scores loop per chunk: S^T = K Q^T (row-pair-packed), exp on ACT, mask
  on DVE, U (+ones column) matmul, row-recips; raw U + recips persist.
  The chunk-probs mix is spread one N=512 slice per scores iteration.
- Tail is a 3-stage software pipeline per chunk: LT matmuls + U*recip+LT
  combine (scalar_tensor_tensor on DVE); PE transposes + attnT copies a
  chunk ahead (so LDWEIGHTS prefetches); output projection with ACT
  evacuation and bf16 output DMA spread across the whole tail.
- Output is written bf16; the host sums partials in f32 and adds bo.
"""

import numpy as np

try:
    import concourse.bass as bass
except ImportError:
    import sys

    sys.path.insert(0, "/opt/trn_rl_repo")
    import concourse.bass as bass

import ml_dtypes
from contextlib import ExitStack

import concourse.tile as tile
from concourse import mybir
from concourse.bass_utils import run_bass_kernel_spmd

BF16 = ml_dtypes.bfloat16
B, S, D, H, CS = 2, 4096, 1024, 16, 128
HD = D // H          # 64
C = S // CS          # 32
NCORES = 8
DHC = 4 * HD         # 256 head dims per core
KB = D // 128        # 8 contraction blocks
NEG = -1e9

bf = mybir.dt.bfloat16
f32 = mybir.dt.float32
Exp = mybir.ActivationFunctionType.Exp
MULT = mybir.AluOpType.mult
ADD = mybir.AluOpType.add


def _split_heavy_waits(nc, keep=1):
    """The walrus build in this container rejects >keep sync waits on several
    instruction encodings. Hoist excess waits onto preceding NoOps on the
    same engine — the sequencer executes them in order."""
    for fn in nc.m.functions:
        for bb in fn.blocks:
            insts = bb.instructions
            i = 0
            while i < len(insts):
                inst = insts[i]
                si = inst.sync_info
                if si is not None and si.on_wait and len(si.on_wait) > keep:
                    waits = list(si.on_wait)
                    head, tail = waits[:-keep], waits[-keep:]
                    for j, w in enumerate(head):
                        nop = mybir.InstNoOp(
                            name=nc.get_next_instruction_name(), ins=[], outs=[]
                        )
                        nop.engine = inst.engine
                        nop.sync_info = mybir.SyncInfo(on_wait=[w], on_update=[])
                        nc.register_instruction(nop, overwrite=True)
                        insts.insert(i + j, nop)
                    inst.sync_info = mybir.SyncInfo(
                        on_wait=tail, on_update=list(si.on_update)
                    )
                    i += len(head)
                i += 1


def _build_program():
    nc = bass.Bass("TRN2", debug=False, num_devices=NCORES)

    ap = {}
    def din(name, shape, dtype):
        ap[name] = nc.dram_tensor(name, shape, dtype, kind="ExternalInput").ap()

    din("xT", [D, S], bf)
    # weights arrive pre-arranged [p, k, d] (host does the permute) so each
    # load is one contiguous-per-partition DMA
    for w in ("wqT", "wrT", "whT"):
        din(w, [128, KB, DHC], bf)
    din("wvkT", [128, KB, 512], bf)
    din("woT", [128, 2, D], bf)
    din("bias", [128, 6], f32)
    din("bvb", [128, 512], bf)      # [bv | bk] broadcast to 128 partitions
    din("ident", [128, 128], bf)
    din("maskb4", [128, 512], bf)   # binary local mask, [t, s] tiled x4 heads
    din("cmask4", [128, C], f32)    # additive chunk mask [c', c] tiled x4
    out_ap = nc.dram_tensor("out", [S, D], bf, kind="ExternalOutput").ap()

    with tile.TileContext(nc) as tc, ExitStack() as ctx:
        const = ctx.enter_context(tc.tile_pool(name="const", bufs=1))
        wpool = ctx.enter_context(tc.tile_pool(name="wpool", bufs=1))
        big = ctx.enter_context(tc.tile_pool(name="big", bufs=1))
        dram = ctx.enter_context(tc.tile_pool(name="dramp", bufs=1, space="DRAM"))

        # ---- weights first (smallest latency to first matmul) ----
        # DMA instruction issue costs ~0.7us of the issuing engine's
        # sequencer and serializes with its compute dispatch. So: wq + the
        # x stream + all mid-kernel DMAs go on Sync (which has no compute);
        # Scalar's queue only carries startup weight/const loads that
        # complete before ACT's first bias add.
        def load_w(name, eng, shape):
            t = wpool.tile(shape, bf, name=f"{name}_sb")
            eng.dma_start(t[:], ap[name][:])
            return t

        # sync queue carries only x (and later feedback DMAs); weights ride
        # scalar so the first matmul's two inputs stream in parallel
        wq_sb = load_w("wqT", nc.scalar, [128, KB, DHC])
        wvk_sb = load_w("wvkT", nc.scalar, [128, KB, 512])
        bvb_sb = const.tile([128, 512], bf)
        nc.scalar.dma_start(bvb_sb[:], ap["bvb"][:])
        bias_sb = const.tile([128, 6], f32)
        nc.scalar.dma_start(bias_sb[:], ap["bias"][:])

        # ---- persistent activations ----
        qt_sb = [big.tile([128, S], bf, name=f"qt{m}") for m in range(2)]
        kt_sb = big.tile([128, 2, S], bf)        # [d-in-m, m, s]
        v_sb = [big.tile([128, 4 * 65], bf, name=f"v{i}") for i in range(C)]
        knat_sb = big.tile([128, C, DHC], bf)    # [s-in-chunk, chunk, d]
        rt_sb = big.tile([128, 2, C], bf)
        ht_sb = big.tile([128, 2, C], bf)
        expct_sb = big.tile([128, C], bf)
        crecip_sb = big.tile([128, 1], f32)
        fnat_sb = [big.tile([128, C * 64], bf, name=f"fnat{p}") for p in range(2)]
        frows_sb = big.tile([128, 64 * 64], bf)
        fb = dram.tile([2, 2, C, 64, 64], bf)    # (pair, h2, c', d, e) c-major
        rbt = dram.tile([4, C, 64, 64], bf)      # (head, c, d, e)
        knd = dram.tile([C, 128, DHC], bf)       # K natural bounce for XBAR T

        onescol_sb = const.tile([128, 1], bf)
        nc.vector.memset(onescol_sb[:], 1.0)
        # ones column per head inside each v tile (static, written once)
        for i in range(C):
            nc.vector.memset(
                v_sb[i].rearrange("p (h e) -> p h e", e=65)[:, :, 64:65], 1.0
            )

        # ---- phase 1: load x, projections ----
        with tc.tile_pool(name="xtp", bufs=1) as xtpool:
            xt_sb = xtpool.tile([128, KB, S], bf)
            # one DMA instruction per s-slice (all k blocks in its AP): the
            # first projection matmuls can start after ~1/8 of x arrives.
            # n=0 is further split so the first chain's k=0..3 deps land
            # ~3us earlier.
            xsrc = ap["xT"].rearrange("(k p) s -> p k s", p=128)
            nc.sync.dma_start(xt_sb[:, 0:4, 0:512], xsrc[:, 0:4, 0:512])
            nc.sync.dma_start(xt_sb[:, 4:8, 0:512], xsrc[:, 4:8, 0:512])
            nc.scalar.dma_start(xt_sb[:, 4:8, 512:1024], xsrc[:, 4:8, 512:1024])
            nc.sync.dma_start(xt_sb[:, 0:4, 512:1024], xsrc[:, 0:4, 512:1024])
            for n in range(2, 8):
                nc.sync.dma_start(
                    xt_sb[:, :, n * 512 : (n + 1) * 512],
                    xsrc[:, :, n * 512 : (n + 1) * 512],
                )

            # low-urgency weights/consts after the x stream, scalar queue
            wr_sb = load_w("wrT", nc.scalar, [128, KB, DHC])
            wh_sb = load_w("whT", nc.scalar, [128, KB, DHC])
            wo_sb = load_w("woT", nc.scalar, [128, 2, D])
            maskb_sb = const.tile([128, 512], bf)
            nc.scalar.dma_start(maskb_sb[:], ap["maskb4"][:])
            cmask_sb = const.tile([128, C], f32)
            nc.scalar.dma_start(cmask_sb[:], ap["cmask4"][:])
            ident_sb = const.tile([128, 128], bf)
            nc.scalar.dma_start(ident_sb[:], ap["ident"][:])

            bvbv = bvb_sb[:, 0:DHC].rearrange("p (h e) -> p h e", e=64)
            knd_pc = knd.rearrange("c p d -> p c d")
            knd_flat = knd.rearrange("c p d -> (c p) d")

            def emit_k_bounce(q):
                # K^T via DRAM bounce + XBAR transpose (no PE cost), in
                # quarters so the transpose engine time hides under the
                # projection matmuls. Rides the (otherwise idle) sync queue.
                c0, c1 = q * 8, q * 8 + 8
                nc.sync.dma_start(
                    knd_pc[:, c0:c1, :], knat_sb[:, c0:c1, :]
                )
                for m in range(2):
                    nc.sync.dma_start_transpose(
                        kt_sb[:, m, c0 * 128 : c1 * 128],
                        knd_flat[c0 * 128 : c1 * 128, m * 128 : (m + 1) * 128],
                    )

            def emit_f_block(fpsp, chunks):
                # F = k^T v for chunks whose knat/v landed last iteration
                for i in chunks:
                    vr = v_sb[i].rearrange("p (h e) -> p h e", e=65)
                    for p in range(2):
                        fps = fpsp.tile([128, 64], f32, tag="fps")
                        for h2 in range(2):
                            nc.tensor.matmul(
                                fps[64 * h2 : 64 * h2 + 64, :],
                                knat_sb[:, i, (2 * p + h2) * 64 : (2 * p + h2) * 64 + 64],
                                vr[:, 2 * p + h2, 0:64],
                                start=True, stop=True, skip_group_check=True,
                            )
                        if (i + p) % 2 == 0:
                            nc.vector.tensor_copy(
                                fnat_sb[p][:, i * 64 : (i + 1) * 64], fps[:]
                            )
                        else:
                            nc.scalar.copy(
                                fnat_sb[p][:, i * 64 : (i + 1) * 64], fps[:]
                            )

            def emit_f_bounce(q):
                # bounce F quarter to row layout (sync queue). The d<->c swap
                # costs ~2k 128B descriptors per quarter no matter what; put
                # that slow side on the WRITE (c-major dst) so it overlaps F
                # and the scores loop, leaving a fast contiguous read.
                c0, c1 = q * 8, q * 8 + 8
                for p in range(2):
                    for h2 in range(2):
                        nc.sync.dma_start(
                            fb[p, h2][c0:c1].rearrange("c d e -> d c e"),
                            fnat_sb[p][64 * h2 : 64 * h2 + 64, c0 * 64 : c1 * 64]
                            .rearrange("d (c e) -> d c e", e=64),
                        )

            def emit_f_rows_read():
                for h in range(4):
                    p, h2 = divmod(h, 2)
                    nc.sync.dma_start(
                        frows_sb[32 * h : 32 * h + 32, :],
                        fb[p, h2].rearrange("c d e -> c (d e)"),
                    )

            with (
                tc.tile_pool(name="pjp", bufs=2, space="PSUM") as pjp,
                tc.tile_pool(name="fps", bufs=2, space="PSUM") as fpsp,
            ):
                for n in range(8):
                    if n == 4:
                        # F for the first half as one dense block: its slow
                        # row-layout bounce (128B descriptors) streams while
                        # the second half of the projections runs
                        for q in range(2):
                            emit_f_block(fpsp, range(q * 8, q * 8 + 8))
                            emit_f_bounce(q)
                    # QT: [dh, s] layout
                    for m in range(2):
                        ps = pjp.tile([128, 512], f32, tag="pj")
                        for k in range(KB):
                            nc.tensor.matmul(
                                ps[:],
                                wq_sb[:, k, m * 128 : (m + 1) * 128],
                                xt_sb[:, k, n * 512 : (n + 1) * 512],
                                start=(k == 0),
                                stop=(k == KB - 1),
                            )
                        nc.scalar.add(
                            qt_sb[m][:, n * 512 : (n + 1) * 512],
                            ps[:],
                            bias_sb[:, m : m + 1],
                        )
                    # V + K natural [s, dh] in one N=512 matmul chain; biases
                    # added during PSUM evacuation from a prebroadcast tile
                    for i in range(4 * n, 4 * n + 4):
                        ps = pjp.tile([128, 512], f32, tag="pv")
                        for k in range(KB):
                            nc.tensor.matmul(
                                ps[:],
                                xt_sb[:, k, i * 128 : (i + 1) * 128],
                                wvk_sb[:, k, :],
                                start=(k == 0),
                                stop=(k == KB - 1),
                                skip_group_check=True,
                            )
                        vr = v_sb[i].rearrange("p (h e) -> p h e", e=65)
                        nc.vector.tensor_add(
                            vr[:, :, 0:64],
                            ps[:, 0:DHC].rearrange("p (h e) -> p h e", e=64),
                            bvbv,
                        )
                        nc.vector.tensor_add(
                            knat_sb[:, i, :], ps[:, DHC:512], bvb_sb[:, DHC:512]
                        )
                    if n % 2 == 1:
                        emit_k_bounce(n // 2)

                # r/h meta projections: [dh, c] layout
                crepr = [
                    xt_sb[:, k, :].rearrange("p (c cs) -> p c cs", cs=CS)[:, :, CS - 1]
                    for k in range(KB)
                ]
                for w_sb, dst, bcol in ((wr_sb, rt_sb, 2), (wh_sb, ht_sb, 4)):
                    for m in range(2):
                        ps = pjp.tile([128, C], f32, tag="pr")
                        for k in range(KB):
                            nc.tensor.matmul(
                                ps[:],
                                w_sb[:, k, m * 128 : (m + 1) * 128],
                                crepr[k],
                                start=(k == 0),
                                stop=(k == KB - 1),
                            )
                        nc.scalar.add(
                            dst[:, m, :], ps[:], bias_sb[:, bcol + m : bcol + m + 1]
                        )

        # phase-2+ tensors reuse the space freed by the xt tiles
        anp2 = ctx.enter_context(tc.tile_pool(name="anp2", bufs=1))
        retrrows_sb = anp2.tile([128, 64 * 64], bf)
        retrt_sb = [anp2.tile([128, C * 64], bf, name=f"retrt{p}") for p in range(2)]
        ubf_sb = [anp2.tile([128, DHC], bf, name=f"ubf{i}") for i in range(C)]
        rr_sb = [anp2.tile([128, 4], f32, name=f"rr{i}") for i in range(C)]

        # ---- F = k^T v per chunk (dense standalone phase right after the
        # projections), with the slow row-layout bounce kicked off per
        # 8-chunk quarter so the gathers overlap the rest of F + scores ----
        with tc.tile_pool(name="fps", bufs=4, space="PSUM") as fpsp:
            for q in range(2, 4):
                emit_f_block(fpsp, range(q * 8, q * 8 + 8))
                emit_f_bounce(q)
            emit_f_rows_read()

        # chunk scores -> probs; crecip (per query chunk) applied on mix output
        with tc.tile_pool(name="csp", bufs=1, space="PSUM") as cspp:
            csp = cspp.tile([128, C], f32, tag="csp")
            for h in range(4):
                hb = 64 * (h % 2)
                nc.tensor.matmul(
                    csp[32 * h : 32 * h + 32, :],
                    ht_sb[hb : hb + 64, h // 2, :],
                    rt_sb[hb : hb + 64, h // 2, :],
                    start=True, stop=True, skip_group_check=True,
                    tile_position=(hb, 32 * h),
                )
            nc.vector.tensor_add(csp[:], csp[:], cmask_sb[:])
            nc.scalar.activation(expct_sb[:], csp[:], Exp, scale=0.125)
            csums = cspp.tile([128, 1], f32, tag="csums")
            for h in range(4):
                nc.tensor.matmul(
                    csums[32 * h : 32 * h + 32, :],
                    expct_sb[32 * h : 32 * h + 32, :],
                    onescol_sb[32 * h : 32 * h + 32, :],
                    start=True, stop=True, skip_group_check=True,
                    tile_position=(32 * h, 32 * h),
                )
            nc.vector.reciprocal(crecip_sb[:], csums[:])

        # ---- scores loop (st/exp/U) overlapping the retrieval bounces ----
        # Concurrent (row-disjoint) PE matmuls must not drain into the
        # same PSUM bank on the same partitions: heads with operands at
        # base 0 (h0/h2) and base 64 (h1/h3) can execute concurrently on
        # disjoint PE row groups, so each group gets its own bank.
        POS = {0: (0, 0), 2: (0, 1), 1: (1, 0), 3: (1, 1)}
        HORD = (0, 2, 1, 3)

        with (
            tc.tile_pool(name="stp", bufs=2, space="PSUM") as stp,
            tc.tile_pool(name="ulp", bufs=2, space="PSUM") as ulp,
            tc.tile_pool(name="exps", bufs=4) as expp,
            tc.tile_pool(name="smalls", bufs=4) as smalls,
        ):
            expst_q = {}

            def emit_st(i):
                st = stp.tile([128, 1024], f32, tag="st")
                for h in HORD:
                    hp, hb = h // 2, 64 * (h % 2)
                    g, b = POS[h]
                    nc.tensor.matmul(
                        st[:, g * 512 + b * 128 : g * 512 + (b + 1) * 128],
                        kt_sb[hb : hb + 64, hp, i * 128 : (i + 1) * 128],
                        qt_sb[hp][hb : hb + 64, i * 128 : (i + 1) * 128],
                        start=(b == 0), stop=(b == 1), skip_group_check=True,
                    )
                stv = st.rearrange("p (g c) -> p g c", c=512)[:, :, 0:256].rearrange(
                    "p g (b e) -> p g b e", e=128
                )
                expraw = expp.tile([128, 512], bf, tag="expraw")
                nc.scalar.activation(expraw[:], stv, Exp, scale=0.125)
                expst = expp.tile([128, 512], bf, tag="expst")
                nc.vector.tensor_mul(expst[:], expraw[:], maskb_sb[:])
                expst_q[i] = expst

            def emit_u(i):
                # trails emit_st by one chunk so the U matmuls' stationary
                # operand (expst) is ready early enough for LDWEIGHTS
                # prefetch
                expst = expst_q.pop(i)
                u = ulp.tile([128, 4 * 65], f32, tag="u")
                for h in range(4):
                    g, b = POS[h]
                    pos = g * 2 + b
                    nc.tensor.matmul(
                        u[:, h * 65 : (h + 1) * 65],
                        expst[:, pos * 128 : (pos + 1) * 128],
                        v_sb[i][:, h * 65 : (h + 1) * 65],
                        start=(h == 0), stop=(h == 3), skip_group_check=True,
                    )
                # row-sum reciprocals straight from the PSUM ones column;
                # U is stored raw (normalization folds into the tail combine)
                nc.vector.reciprocal(
                    rr_sb[i][:], u.rearrange("p (h e) -> p h e", e=65)[:, :, 64]
                )
                nc.vector.tensor_copy(
                    ubf_sb[i].rearrange("p (h e) -> p h e", e=64),
                    u.rearrange("p (h e) -> p h e", e=65)[:, :, 0:64],
                )

            for i in range(C):
                emit_st(i)
                if i >= 1:
                    emit_u(i - 1)
                if i == C - 1:
                    emit_u(i)
                if 6 <= i < 14:
                    # retrieved rows = probs @ F rows, one N=512 slice per
                    # scores iteration (spreads PE/ACT/DVE load and rides
                    # out the F-bounce latency)
                    nb = i - 6
                    mps = ulp.tile([128, 512], f32, tag="mps")
                    for h in range(4):
                        nc.tensor.matmul(
                            mps[32 * h : 32 * h + 32, :],
                            expct_sb[32 * h : 32 * h + 32, :],
                            frows_sb[32 * h : 32 * h + 32, nb * 512 : (nb + 1) * 512],
                            start=True, stop=True, skip_group_check=True,
                            tile_position=(32 * h, 32 * h),
                        )
                    if nb % 2 == 0:
                        nc.scalar.mul(
                            retrrows_sb[:, nb * 512 : (nb + 1) * 512],
                            mps[:],
                            crecip_sb[:, 0:1],
                        )
                    else:
                        nc.vector.tensor_scalar_mul(
                            retrrows_sb[:, nb * 512 : (nb + 1) * 512],
                            mps[:],
                            crecip_sb[:, 0:1],
                        )
                if i == 14:
                    # retrieved rows -> retrT via DRAM bounce (sync queue is
                    # idle between the K^T transposes and the output writes).
                    # The reads come back per 8-chunk quarter so the tail's
                    # first chunks unblock ~3 gathers earlier.
                    for h in range(4):
                        nc.sync.dma_start(
                            rbt[h],
                            retrrows_sb[32 * h : 32 * h + 32, :].rearrange(
                                "c (d e) -> c d e", e=64
                            ),
                        )
                    for cq in range(4):
                        c0, c1 = cq * 8, cq * 8 + 8
                        for p in range(2):
                            for h2 in range(2):
                                nc.sync.dma_start(
                                    retrt_sb[p][
                                        64 * h2 : 64 * h2 + 64, c0 * 64 : c1 * 64
                                    ].rearrange("d (c e) -> d c e", e=64),
                                    rbt[2 * p + h2][c0:c1].rearrange("c d e -> d c e"),
                                )

        # ---- tail loop: LT + combine for chunk i, transpose + output
        # projection + DMA for chunk i-1 (software-pipelined so the PE never
        # waits on the DVE combine or the attnT evacuation round trips) ----
        with (
            tc.tile_pool(name="ltp", bufs=2, space="PSUM") as ltp,
            tc.tile_pool(name="tpp", bufs=1, space="PSUM") as tpp,
            tc.tile_pool(name="outp", bufs=3, space="PSUM") as outp,
            tc.tile_pool(name="attns", bufs=4) as attns,
            tc.tile_pool(name="outs", bufs=4) as outs,
        ):
            attn_q = {}
            attnT_q = {}

            def emit_lt(i):
                lt = ltp.tile([128, 1024], f32, tag="lt")
                for h in HORD:
                    hp, hb = h // 2, 64 * (h % 2)
                    g, b = POS[h]
                    nc.tensor.matmul(
                        lt[:, g * 512 + b * 64 : g * 512 + (b + 1) * 64],
                        qt_sb[hp][hb : hb + 64, i * 128 : (i + 1) * 128],
                        retrt_sb[hp][hb : hb + 64, i * 64 : (i + 1) * 64],
                        start=(b == 0), stop=(b == 1), skip_group_check=True,
                    )
                # attn = U * rowrecip + LT per head on DVE
                attn = attns.tile([128, DHC], bf, tag="attn")
                ur = ubf_sb[i].rearrange("p (h e) -> p h e", e=64)
                for h in range(4):
                    g, b = POS[h]
                    nc.vector.scalar_tensor_tensor(
                        attn[:, h * 64 : (h + 1) * 64],
                        ur[:, h, :],
                        rr_sb[i][:, h : h + 1],
                        lt[:, g * 512 + b * 64 : g * 512 + (b + 1) * 64],
                        op0=MULT,
                        op1=ADD,
                    )
                attn_q[i] = attn

            def emit_transpose(j):
                attn = attn_q.pop(j)
                attnT = attns.tile([128, 2, 128], bf, tag="attnT")
                for half in range(2):
                    tp = tpp.tile([128, 128], bf, tag="tp")
                    nc.tensor.transpose(
                        tp[:], attn[:, half * 128 : (half + 1) * 128], ident_sb[:]
                    )
                    nc.vector.tensor_copy(attnT[:, half, :], tp[:])
                attnT_q[j] = attnT

            def emit_out(j):
                attnT = attnT_q.pop(j)
                osb = outs.tile([128, D], bf, tag="osb")
                for nb in range(2):
                    ops = outp.tile([128, 512], f32, tag="ops")
                    for p in range(2):
                        nc.tensor.matmul(
                            ops[:],
                            attnT[:, p, :],
                            wo_sb[:, p, nb * 512 : (nb + 1) * 512],
                            start=(p == 0),
                            stop=(p == 1),
                        )
                    if nb == 0 and j >= C - 2:
                        # drain: nothing follows, use both engines in parallel
                        nc.vector.tensor_copy(osb[:, 0:512], ops[:])
                    else:
                        nc.scalar.copy(osb[:, nb * 512 : (nb + 1) * 512], ops[:])
                nc.sync.dma_start(out_ap[j * 128 : (j + 1) * 128, :], osb[:])

            # 3-stage pipeline: the attnT copies lead their output projection
            # by a full chunk so the PE's weight load can prefetch
            for i in range(C):
                emit_lt(i)
                if i >= 1:
                    emit_transpose(i - 1)
                if i >= 2:
                    emit_out(i - 2)
            emit_transpose(C - 1)
            emit_out(C - 2)
            emit_out(C - 1)

    _split_heavy_waits(nc)
    return nc


_CACHE = {}


def _get_program():
    if "nc" not in _CACHE:
        _CACHE["nc"] = _build_program()
    return _CACHE["nc"]


def _make_in_maps(inputs):
    hs = np.asarray(inputs["hidden_states"], dtype=np.float32)
    W = {k: np.asarray(inputs[k], dtype=np.float32) for k in
         ("Wq", "Wk", "Wv", "Wo", "Wr", "Wh")}
    bvec = {k: np.asarray(inputs[k], dtype=np.float32) for k in
            ("bq", "bk", "bv", "bo", "br", "bh")}

    # local binary mask in [t, s] layout (keep t >= s), tiled x4 heads
    tt, ss = np.meshgrid(np.arange(128), np.arange(128), indexing="ij")
    maskb = (tt >= ss).astype(np.float32)
    maskb4 = np.tile(maskb, (1, 4)).astype(BF16)
    # chunk mask in [c', c] layout: keep c' >= c; tiled x4 heads (additive)
    cc2, cc = np.meshgrid(np.arange(C), np.arange(C), indexing="ij")
    cmask = np.where(cc2 >= cc, 0.0, NEG).astype(np.float32)
    cmask4 = np.tile(cmask, (4, 1)).astype(np.float32)
    ident = np.eye(128, dtype=np.float32).astype(BF16)

    xT_bf = [np.ascontiguousarray(hs[b].T).astype(BF16) for b in range(B)]

    in_maps = []
    for c in range(NCORES):
        b, hg = divmod(c, 4)
        sl = slice(hg * DHC, (hg + 1) * DHC)
        bias = np.stack(
            [
                bvec["bq"][sl][:128], bvec["bq"][sl][128:],
                bvec["br"][sl][:128], bvec["br"][sl][128:],
                bvec["bh"][sl][:128], bvec["bh"][sl][128:],
            ],
            axis=1,
        ).astype(np.float32)
        bvb = np.tile(
            np.concatenate([bvec["bv"][sl], bvec["bk"][sl]])[None, :], (128, 1)
        ).astype(BF16)

        # pre-arrange weights to the device's [p, k, d] layout so each load
        # is a single contiguous-per-partition DMA
        def arr(wT):  # wT: [D, cols] -> [128, KB, cols]
            return np.ascontiguousarray(
                wT.reshape(KB, 128, wT.shape[1]).transpose(1, 0, 2)
            ).astype(BF16)

        wvk = np.concatenate([W["Wv"][sl, :].T, W["Wk"][sl, :].T], axis=1)
        woT = W["Wo"][:, sl].T  # [DHC, D]
        in_maps.append(
            {
                "xT": xT_bf[b],
                "wqT": arr(W["Wq"][sl, :].T),
                "wvkT": arr(wvk),
                "wrT": arr(W["Wr"][sl, :].T),
                "whT": arr(W["Wh"][sl, :].T),
                "woT": np.ascontiguousarray(
                    woT.reshape(2, 128, D).transpose(1, 0, 2)
                ).astype(BF16),
                "bias": bias,
                "bvb": bvb,
                "ident": ident,
                "maskb4": maskb4,
                "cmask4": cmask4,
            }
        )
    return in_maps, bvec["bo"]


def kernel(**inputs):
    nc = _get_program()
    in_maps, bo = _make_in_maps(inputs)
    res = run_bass_kernel_spmd(nc, in_maps, core_ids=list(range(NCORES)))
    _CACHE["last_results"] = res
    out = np.zeros((B, S, D), np.float32)
    for c in range(NCORES):
        out[c // 4] += res.results[c]["out"].astype(np.float32)
    out += bo[None, None, :]
    return out
